# revision 27
# baseline (speedup 1.0000x reference)
"""Trainium2 Bass kernel for MinimalConvWTA_LIF.

Model: u = three causal convs (k=8/16/32, scaled 1/sqrt(k)) over x[B,1,T];
s = winner-take-all LIF spike train over u with alpha=0.95, theta=1.0.

Strategy (per NeuronCore, pure data parallel over batch, 32 rows/core):
  * conv: PE matmuls against host-built banded weight matrices, 4 windows
    packed per matmul pair (full 128 stationary columns).
  * LIF scan: time split into 128 chunks of C=128, all advanced in a
    wavefront.  SBUF layout [128 partitions = 32 batch x 4 chunk-slots],
    free = [Q=32 step-quarter, NC2=32 chunks, 4 lanes(3 used)].  One step is
    3 DVE ops over every chunk:
       1. v = alpha*v_prev + u          (scalar_tensor_tensor)
       2. g = max over the 3 channels   (tensor_reduce)
       3. v = v - (v >= max(g, theta))  (custom DVE op LIF_RESET_ANT)
    The v trajectory is kept (vq tiles); spikes are recovered in bulk at
    the end as s = (alpha*v[t-1] + u[t]) - v[t], written over the u tiles.
  * chunk boundary states are resolved by iteration: P=3 passes; pass p+1
    starts every chunk from the end state of its left neighbour in pass p
    (alpha^256 contraction => a handful of spike flips globally).
"""

import sys

import numpy as np

_TRN_REPO = "/opt/trn_rl_repo"
if _TRN_REPO not in sys.path:
    sys.path.insert(0, _TRN_REPO)

import concourse.bass as bass
import concourse.mybir as mybir
from concourse import bacc, tile
from concourse.bass_utils import run_bass_kernel_spmd
import concourse.dve_ops as dve_ops_mod
from concourse.dve_ops import DveOp
from concourse.dve_spec import Spec, Src0, Src1, C0, maxx, lower
from concourse.dve_uop import DveOpSpec

# ---------------------------------------------------------------- constants
B_FULL = 256
T_FULL = 16384
N_CORES = 8
ALPHA = np.float32(0.95)
F32 = mybir.dt.float32
A = mybir.AluOpType
SCAT_POOL = False

Bc = 32          # batch rows per core
CS = 4           # chunk slots along partitions
C = 128          # chunk length (timesteps)
NC2 = 32         # chunks along the free dim (T/(C*CS))
NQ = 4
Q = C // NQ      # 32
P = 3            # boundary-iteration passes
NW = T_FULL // 128   # conv output blocks = chunks
WIN_OUT = 128
LPAD = 128
XTILES = NW + 1      # 129 transposed x tiles (one leading zero tile)
NE = (XTILES + 1) // 2
NO = XTILES // 2


# ------------------------------------------------------- custom DVE ops
def _register(name, spec):
    if name in dve_ops_mod._SUB_OPCODE_FOR_NAME:
        return next(o for o in dve_ops_mod.OPS if o.name == name)
    row = dve_ops_mod._CUSTOM_DVE_ROW_BASE + len(dve_ops_mod.OPS)
    assert row < 0x20
    shas = {}
    for ver in ("v3", "v4"):
        try:
            s = DveOpSpec(name=name, opcode=row, uops=lower(spec, ver=ver),
                          rd1_en=True)
            shas[ver] = s.sha(ver)
        except Exception:
            pass
    op = DveOp(name, spec, subdim=False, uops_sha=shas)
    dve_ops_mod.OPS.append(op)
    dve_ops_mod._SUB_OPCODE_FOR_NAME[name] = row
    dve_ops_mod.CUSTOM_DVE_SPECS[name] = spec
    return op


# v_post = v - (v >= max(g, theta)); s0 = theta
LIF_RESET = _register("LIF_RESET_ANT", Spec(
    body=Src0 - (Src0 >= maxx(Src1, C0)),
    reference=lambda in0, in1, s0, s1, imm2:
        (in0 - (in0 >= np.maximum(in1, s0))).astype(np.float32),
))


# ------------------------------------------------------------- host helpers
def build_walls(ws):
    """Banded conv-weight matrices, quarter/lane-blocked columns:
    wallA [128, 96] col = k*32+t  (t<32); wallB [128, 4*96] col = q*96+k*32+t'."""
    wallA = np.zeros((128, 96), np.float32)
    wallB = np.zeros((128, 4 * 96), np.float32)
    for k, w in enumerate(ws):
        kl = len(w)
        scale = np.float32(1.0 / np.sqrt(np.float32(kl)))
        wk = (w.astype(np.float32) * scale).astype(np.float32)
        for tl in range(WIN_OUT):
            q, tq = divmod(tl, 32)
            for d in range(kl):
                rA = tl + 128 - d
                if 64 <= rA < 128 and tl < 32:
                    wallA[rA, k * 32 + tl] = wk[kl - 1 - d]
                rB = tl - d
                if 0 <= rB < 128:
                    wallB[rB, q * 96 + k * 32 + tq] = wk[kl - 1 - d]
    return wallA, wallB


# strip block order: window group g(c2) = {32*cs + c2} needs its 4 A-tiles
# {c2, c2+32, c2+64, c2+96} and B-tiles {c2+1, ...} each contiguous.
# Even strip blocks: c2p in (0, 2, ..., 30, 32); odd strip: c2p in (1, 3, .., 31).
EVEN_BLOCKS = list(range(0, 31, 2)) + [32]
ODD_BLOCKS = list(range(1, 32, 2))
NE_POS = 4 * len(EVEN_BLOCKS)
NO_POS = 4 * len(ODD_BLOCKS)
# block start position (in tiles) of the block whose first tile is c2p
EVEN_POS = {c2p: 4 * i for i, c2p in enumerate(EVEN_BLOCKS)}
ODD_POS = {c2p: 4 * i for i, c2p in enumerate(ODD_BLOCKS)}


def build_xt(x2d):
    """Host-side transposed x strips in block order: block (c2p) holds tiles
    {c2p, c2p+32, c2p+64, c2p+96} of xp = [128 zeros] + x, each transposed
    to [128 time, 32 batch]."""
    Bb = x2d.shape[0]
    xp = np.zeros((Bb, LPAD + T_FULL), np.float32)
    xp[:, LPAD:] = x2d
    t = np.zeros((Bb, XTILES + 1, 128), np.float32)
    t[:, :XTILES] = xp.reshape(Bb, XTILES, 128)   # tile 129 stays zero (unused)
    t = t.transpose(2, 1, 0)                      # [128, XTILES+1, Bb]
    xte = np.zeros((128, NE_POS, Bb), np.float32)
    for i, c2p in enumerate(EVEN_BLOCKS):
        xte[:, 4 * i:4 * i + 4] = t[:, [c2p, c2p + 32, c2p + 64, c2p + 96]]
    xto = np.zeros((128, NO_POS, Bb), np.float32)
    for i, c2p in enumerate(ODD_BLOCKS):
        xto[:, 4 * i:4 * i + 4] = t[:, [c2p, c2p + 32, c2p + 64, c2p + 96]]
    return (np.ascontiguousarray(xte).reshape(128, NE_POS * Bb),
            np.ascontiguousarray(xto).reshape(128, NO_POS * Bb))


# ------------------------------------------------------------ program build
def build_program():
    nc = bacc.Bacc("TRN2", target_bir_lowering=False, debug=False)

    xte_d = nc.dram_tensor("xte", [128, NE_POS * Bc], F32, kind="ExternalInput")
    xto_d = nc.dram_tensor("xto", [128, NO_POS * Bc], F32, kind="ExternalInput")
    wa_d = nc.dram_tensor("wallA", [128, 96], F32, kind="ExternalInput")
    wb_d = nc.dram_tensor("wallB", [128, 4 * 96], F32, kind="ExternalInput")
    u_d = nc.dram_tensor("u_out", [Bc, 3, T_FULL], F32, kind="ExternalOutput")
    s_d = nc.dram_tensor("s_out", [Bc, 3, T_FULL], F32, kind="ExternalOutput")

    with tile.TileContext(nc) as tc:
        with (
            tc.tile_pool(name="const", bufs=1) as constp,
            tc.tile_pool(name="xbuf", bufs=1) as xbuf,
            tc.tile_pool(name="wave", bufs=1) as wave,
            tc.tile_pool(name="state", bufs=1) as state,
            tc.tile_pool(name="psC", bufs=8, space="PSUM") as psC,
        ):
            wa_sb = constp.tile([128, 96], F32, tag="wa")
            wb_sb = constp.tile([128, 4 * 96], F32, tag="wb")
            xTe = xbuf.tile([128, NE_POS, Bc], F32, tag="xTe")
            xTo = xbuf.tile([128, NO_POS, Bc], F32, tag="xTo")
            nc.sync.dma_start(wa_sb[:], wa_d.ap())
            nc.sync.dma_start(wb_sb[:], wb_d.ap())
            # split strip loads so early matmuls can start promptly
            nxd = 4
            for i in range(nxd):
                el = NE_POS * Bc // nxd
                ol = NO_POS * Bc // nxd
                nc.sync.dma_start(xTe[:, 0, 0].tensor_slice2(i * el, el),
                                  xte_d.ap()[:, i * el:(i + 1) * el]) \
                    if False else None
                e0 = xTe[:, 0, :]
                nc.sync.dma_start(
                    bass.AP(e0.tensor, e0.offset + i * el, [e0.ap[0], [1, el]]),
                    xte_d.ap()[:, i * el:(i + 1) * el])
                o0 = xTo[:, 0, :]
                nc.sync.dma_start(
                    bass.AP(o0.tensor, o0.offset + i * ol, [o0.ap[0], [1, ol]]),
                    xto_d.ap()[:, i * ol:(i + 1) * ol])

            def xt_block(c2p, rows=None):
                """Stationary AP for block {c2p, c2p+32, c2p+64, c2p+96}."""
                if c2p % 2 == 0:
                    strip, pos = xTe, EVEN_POS[c2p]
                else:
                    strip, pos = xTo, ODD_POS[c2p]
                a = strip[:, 0, :] if rows is None else strip[rows[0]:rows[1], 0, :]
                return bass.AP(a.tensor, a.offset + pos * Bc,
                               [a.ap[0], [1, 4 * Bc]])

            # wavefront tiles: [128, Q(jq), NC2(c2), 4 lanes (3 used)]
            uq = [wave.tile([128, NC2, 3, Q], F32, tag=f"uq{q}", name=f"uq{q}")
                  for q in range(NQ)]
            vq = [wave.tile([128, Q, NC2, 4], F32, tag=f"vq{q}", name=f"vq{q}")
                  for q in range(NQ)]
            g = state.tile([128, NC2], F32, tag="g")
            binit = state.tile([128, 4], F32, tag="binit")
            vpre0 = state.tile([128, NC2, 4], F32, tag="vpre0")
            alpha_t = wave.tile([128, Q, NC2], F32, tag="alpha_t")
            scr = wave.tile([128, Q, NC2], F32, tag="scr")
            nc.vector.memset(binit[:], 0.0)
            nc.gpsimd.memset(alpha_t[:], float(ALPHA))

            def g_b():
                ga = g[:, :]
                return bass.AP(ga.tensor, ga.offset, list(ga.ap) + [[0, 3]])

            # ---------------- conv: quarter-major matmuls, partition-aligned
            # group g = c2: windows {32*cs + c2}; pc partitions = 32*cs + b
            # align 1:1 with uq partitions -> one [128, 96] copy per
            # (quarter, group).
            for q in range(NQ):
                for c2 in range(NC2):
                    pc = psC.tile([128, 96], F32, tag="psC")
                    lhsB = xt_block(c2 + 1)
                    if q == 0:
                        lhsA = xt_block(c2, rows=(64, 128))
                        nc.tensor.matmul(pc[:], lhsB, wb_sb[:, 0:96],
                                         start=True, stop=False)
                        nc.tensor.matmul(pc[:], lhsA, wa_sb[64:128, :],
                                         start=False, stop=True)
                    else:
                        nc.tensor.matmul(pc[:], lhsB,
                                         wb_sb[:, 96 * q:96 * (q + 1)],
                                         start=True, stop=True)
                    d0 = uq[q][:, c2, 0, 0]
                    dst = bass.AP(d0.tensor, d0.offset, [d0.ap[0], [1, 96]])
                    nc.scalar.copy(dst, pc[:])

            # ---------------- u DMA out: t = 128*(cs*NC2+c2) + q*32 + jq
            for cs in range(CS):
                for q in range(NQ):
                    for k in range(3):
                        src = uq[q][Bc * cs:Bc * (cs + 1), :, k, :]
                        dst_ap = bass.AP(
                            u_d.ap().tensor,
                            (k * T_FULL + cs * NC2 * C + q * Q),
                            [[3 * T_FULL, Bc], [C, NC2], [1, Q]])
                        nc.sync.dma_start(dst_ap, src)

            # ---------------- LIF wavefront
            def vsl(sigma, c2a=0, c2b=NC2):
                q, jq = divmod(sigma, Q)
                return vq[q][:, jq, c2a:c2b, 0:3]

            def usl(sigma, c2a=0, c2b=NC2):
                q, jq = divmod(sigma, Q)
                return uq[q][:, c2a:c2b, :, jq]

            def recover_quarter(q):
                """s = (alpha*v[t-1] + u[t]) - v[t] for quarter q, written
                over uq[q] (Pool engine), then DMA s out."""
                if q == 0:
                    nc.vector.tensor_copy(uq[0][:, :, :, 0], vpre0[:, :, 0:3])
                else:
                    nc.vector.scalar_tensor_tensor(
                        uq[q][:, :, :, 0], vq[q - 1][:, Q - 1, :, 0:3],
                        float(ALPHA), uq[q][:, :, :, 0],
                        op0=A.mult, op1=A.add)
                for k in range(3):
                    v0 = vq[q][:, 0, 0, k]
                    vslab0 = bass.AP(v0.tensor, v0.offset,
                                     [v0.ap[0], [4 * NC2, Q - 1], [4, NC2]])
                    vslab = bass.AP(v0.tensor, v0.offset,
                                    [v0.ap[0], [4 * NC2, Q], [4, NC2]])
                    u0 = uq[q][:, 0, k, 0]
                    uslab1 = bass.AP(u0.tensor, u0.offset + 1,
                                     [u0.ap[0], [1, Q - 1], [3 * Q, NC2]])
                    uslab = bass.AP(u0.tensor, u0.offset,
                                    [u0.ap[0], [1, Q], [3 * Q, NC2]])
                    s0 = scr[:, 0, 0]
                    scrslab0 = bass.AP(s0.tensor, s0.offset,
                                       [s0.ap[0], [NC2, Q - 1], [1, NC2]])
                    a0 = alpha_t[:, 0, 0]
                    aslab0 = bass.AP(a0.tensor, a0.offset,
                                     [a0.ap[0], [NC2, Q - 1], [1, NC2]])
                    nc.gpsimd.tensor_tensor(scrslab0, vslab0, aslab0,
                                            op=A.mult)
                    nc.gpsimd.tensor_tensor(uslab1, uslab1, scrslab0,
                                            op=A.add)
                    nc.gpsimd.tensor_tensor(uslab, uslab, vslab,
                                            op=A.subtract)
                for cs in range(CS):
                    for k in range(3):
                        src = uq[q][Bc * cs:Bc * (cs + 1), :, k, :]
                        dst_ap = bass.AP(
                            s_d.ap().tensor,
                            (k * T_FULL + cs * NC2 * C + q * Q),
                            [[3 * T_FULL, Bc], [C, NC2], [1, Q]])
                        nc.sync.dma_start(dst_ap, src)

            for p in range(P):
                if p > 0:
                    # binit rows 32..127 <- prev-pass end of last chunk of
                    # previous slot (partition-shifted copy); rows 0..31 stay 0
                    for cs in range(1, CS):
                        bsrc0 = vq[NQ - 1][Bc * (cs - 1):Bc * cs,
                                           Q - 1, NC2 - 1, 0:3]
                        bsrc = bass.AP(bsrc0.tensor, bsrc0.offset,
                                       [bsrc0.ap[0], [1, 3]])
                        nc.vector.tensor_copy(binit[Bc * cs:Bc * (cs + 1), 0:3],
                                              bsrc)
                for sigma in range(C):
                    cur = vsl(sigma)
                    if sigma == 0:
                        if p == 0:
                            nc.vector.tensor_copy(cur, usl(0))
                        else:
                            nc.vector.scalar_tensor_tensor(
                                vsl(0, 1, NC2), vq[NQ - 1][:, Q - 1, 0:NC2 - 1, 0:3],
                                float(ALPHA), usl(0, 1, NC2),
                                op0=A.mult, op1=A.add)
                            bi = binit[:, 0:3]
                            bi3 = bass.AP(bi.tensor, bi.offset,
                                          [bi.ap[0], [0, 1], [1, 3]])
                            nc.vector.scalar_tensor_tensor(
                                vsl(0, 0, 1), bi3, float(ALPHA), usl(0, 0, 1),
                                op0=A.mult, op1=A.add)
                    else:
                        nc.vector.scalar_tensor_tensor(
                            cur, vsl(sigma - 1), float(ALPHA), usl(sigma),
                            op0=A.mult, op1=A.add)
                    if p == P - 1 and sigma == 0:
                        nc.vector.tensor_copy(vpre0[:, :, 0:3], cur)
                    nc.vector.tensor_reduce(
                        g[:, :], cur, axis=mybir.AxisListType.X, op=A.max)
                    nc.vector._custom_dve(LIF_RESET, out=cur, in0=cur,
                                          in1=g_b(), s0=1.0)
                    if p == P - 1 and sigma % Q == Q - 1:
                        recover_quarter(sigma // Q)

            # (recovery + s DMA are emitted per-quarter inside the final
            # pass via recover_quarter)

    nc.compile()
    return nc


# ----------------------------------------------------------------- running
def _ensure_ntff_hook():
    """Register the axon NTFF profiling hook."""
    import types
    try:
        from antenv.axon_hooks import get_axon_ntff_profile_hook  # noqa: F401
        return
    except ImportError:
        pass
    import antenv
    mod = types.ModuleType("antenv.axon_hooks")
    _state = {"hook": None}
    mod.set_axon_ntff_profile_hook = lambda h: _state.__setitem__("hook", h)
    mod.get_axon_ntff_profile_hook = lambda: _state["hook"]
    sys.modules["antenv.axon_hooks"] = mod
    antenv.axon_hooks = mod
    try:
        from trn_agent_boot.trn_boot import _ntff_profile_via_ctypes
        hook = _ntff_profile_via_ctypes("/opt/axon/libaxon_pjrt.so")
        if hook is not None:
            mod.set_axon_ntff_profile_hook(hook)
    except Exception as e:  # profiling optional
        print(f"ntff hook unavailable: {e}", file=sys.stderr)


_CACHE = {}


def _get_program():
    if "p" not in _CACHE:
        _CACHE["p"] = build_program()
    return _CACHE["p"]


def kernel(x, w0, w1, w2, y=None, trace=False):
    x = np.asarray(x, np.float32)
    ws = [np.asarray(w, np.float32).reshape(-1) for w in (w0, w1, w2)]
    B = x.shape[0]
    assert B == B_FULL and x.shape[-1] == T_FULL

    wallA, wallB = build_walls(ws)
    xts = [build_xt(x.reshape(B, T_FULL)[c * Bc:(c + 1) * Bc])
           for c in range(N_CORES)]

    if trace:
        _ensure_ntff_hook()
    nc = _get_program()
    in_maps = [
        {"xte": xts[c][0], "xto": xts[c][1], "wallA": wallA, "wallB": wallB}
        for c in range(N_CORES)
    ]
    res = run_bass_kernel_spmd(nc, in_maps, core_ids=list(range(N_CORES)),
                               trace=trace)
    u = np.concatenate([r["u_out"] for r in res.results], axis=0)
    s = np.concatenate([r["s_out"] for r in res.results], axis=0)
    if trace:
        kernel.last_exec_time_ns = res.exec_time_ns
    return (u, s)


kernel.last_exec_time_ns = None


# revision 29
# speedup vs baseline: 1.2170x; 1.2170x over previous
"""Trainium2 Bass kernel for MinimalConvWTA_LIF.

Model: u = three causal convs (k=8/16/32, scaled 1/sqrt(k)) over x[B,1,T];
s = winner-take-all LIF spike train over u with alpha=0.95, theta=1.0.

Strategy (per NeuronCore, pure data parallel over batch, 32 rows/core):
  * conv: PE matmuls against host-built banded weight matrices, 4 windows
    packed per matmul pair (full 128 stationary columns).
  * LIF scan: time split into 128 chunks of C=128, all advanced in a
    wavefront.  SBUF layout [128 partitions = 32 batch x 4 chunk-slots],
    free = [Q=32 step-quarter, NC2=32 chunks, 4 lanes(3 used)].  One step is
    3 DVE ops over every chunk:
       1. v = alpha*v_prev + u          (scalar_tensor_tensor)
       2. g = max over the 3 channels   (tensor_reduce)
       3. v = v - (v >= max(g, theta))  (custom DVE op LIF_RESET_ANT)
    The v trajectory is kept (vq tiles); spikes are recovered in bulk at
    the end as s = (alpha*v[t-1] + u[t]) - v[t], written over the u tiles.
  * chunk boundary states are resolved by iteration: P=3 passes; pass p+1
    starts every chunk from the end state of its left neighbour in pass p
    (alpha^256 contraction => a handful of spike flips globally).
"""

import sys

import numpy as np

_TRN_REPO = "/opt/trn_rl_repo"
if _TRN_REPO not in sys.path:
    sys.path.insert(0, _TRN_REPO)

import concourse.bass as bass
import concourse.mybir as mybir
from concourse import bacc, tile
from concourse.bass_utils import run_bass_kernel_spmd
import concourse.dve_ops as dve_ops_mod
from concourse.dve_ops import DveOp
from concourse.dve_spec import Spec, Src0, Src1, C0, maxx, lower
from concourse.dve_uop import DveOpSpec

# ---------------------------------------------------------------- constants
B_FULL = 256
T_FULL = 16384
N_CORES = 8
ALPHA = np.float32(0.95)
F32 = mybir.dt.float32
A = mybir.AluOpType
SCAT_POOL = False

Bc = 32          # batch rows per core
CS = 4           # chunk slots along partitions
C = 128          # chunk length (timesteps)
NC2 = 32         # chunks along the free dim (T/(C*CS))
NQ = 4
Q = C // NQ      # 32
P = 3            # boundary-iteration passes
H = 96           # steps re-run in the final pass (tail spliced from pass P-1)
NW = T_FULL // 128   # conv output blocks = chunks
WIN_OUT = 128
LPAD = 128
XTILES = NW + 1      # 129 transposed x tiles (one leading zero tile)
NE = (XTILES + 1) // 2
NO = XTILES // 2


# ------------------------------------------------------- custom DVE ops
def _register(name, spec):
    if name in dve_ops_mod._SUB_OPCODE_FOR_NAME:
        return next(o for o in dve_ops_mod.OPS if o.name == name)
    row = dve_ops_mod._CUSTOM_DVE_ROW_BASE + len(dve_ops_mod.OPS)
    assert row < 0x20
    shas = {}
    for ver in ("v3", "v4"):
        try:
            s = DveOpSpec(name=name, opcode=row, uops=lower(spec, ver=ver),
                          rd1_en=True)
            shas[ver] = s.sha(ver)
        except Exception:
            pass
    op = DveOp(name, spec, subdim=False, uops_sha=shas)
    dve_ops_mod.OPS.append(op)
    dve_ops_mod._SUB_OPCODE_FOR_NAME[name] = row
    dve_ops_mod.CUSTOM_DVE_SPECS[name] = spec
    return op


# v_post = v - (v >= max(g, theta)); s0 = theta
LIF_RESET = _register("LIF_RESET_ANT", Spec(
    body=Src0 - (Src0 >= maxx(Src1, C0)),
    reference=lambda in0, in1, s0, s1, imm2:
        (in0 - (in0 >= np.maximum(in1, s0))).astype(np.float32),
))


# ------------------------------------------------------------- host helpers
def build_walls(ws):
    """Banded conv-weight matrices, quarter/lane-blocked columns:
    wallA [128, 96] col = k*32+t  (t<32); wallB [128, 4*96] col = q*96+k*32+t'."""
    wallA = np.zeros((128, 96), np.float32)
    wallB = np.zeros((128, 4 * 96), np.float32)
    for k, w in enumerate(ws):
        kl = len(w)
        scale = np.float32(1.0 / np.sqrt(np.float32(kl)))
        wk = (w.astype(np.float32) * scale).astype(np.float32)
        for tl in range(WIN_OUT):
            q, tq = divmod(tl, 32)
            for d in range(kl):
                rA = tl + 128 - d
                if 64 <= rA < 128 and tl < 32:
                    wallA[rA, k * 32 + tl] = wk[kl - 1 - d]
                rB = tl - d
                if 0 <= rB < 128:
                    wallB[rB, q * 96 + k * 32 + tq] = wk[kl - 1 - d]
    return wallA, wallB


# strip block order: window group g(c2) = {32*cs + c2} needs its 4 A-tiles
# {c2, c2+32, c2+64, c2+96} and B-tiles {c2+1, ...} each contiguous.
# Even strip blocks: c2p in (0, 2, ..., 30, 32); odd strip: c2p in (1, 3, .., 31).
EVEN_BLOCKS = list(range(0, 31, 2)) + [32]
ODD_BLOCKS = list(range(1, 32, 2))
NE_POS = 4 * len(EVEN_BLOCKS)
NO_POS = 4 * len(ODD_BLOCKS)
# block start position (in tiles) of the block whose first tile is c2p
EVEN_POS = {c2p: 4 * i for i, c2p in enumerate(EVEN_BLOCKS)}
ODD_POS = {c2p: 4 * i for i, c2p in enumerate(ODD_BLOCKS)}


def build_xt(x2d):
    """Host-side transposed x strips in block order: block (c2p) holds tiles
    {c2p, c2p+32, c2p+64, c2p+96} of xp = [128 zeros] + x, each transposed
    to [128 time, 32 batch]."""
    Bb = x2d.shape[0]
    xp = np.zeros((Bb, LPAD + T_FULL), np.float32)
    xp[:, LPAD:] = x2d
    t = np.zeros((Bb, XTILES + 1, 128), np.float32)
    t[:, :XTILES] = xp.reshape(Bb, XTILES, 128)   # tile 129 stays zero (unused)
    t = t.transpose(2, 1, 0)                      # [128, XTILES+1, Bb]
    xte = np.zeros((128, NE_POS, Bb), np.float32)
    for i, c2p in enumerate(EVEN_BLOCKS):
        xte[:, 4 * i:4 * i + 4] = t[:, [c2p, c2p + 32, c2p + 64, c2p + 96]]
    xto = np.zeros((128, NO_POS, Bb), np.float32)
    for i, c2p in enumerate(ODD_BLOCKS):
        xto[:, 4 * i:4 * i + 4] = t[:, [c2p, c2p + 32, c2p + 64, c2p + 96]]
    return (np.ascontiguousarray(xte).reshape(128, NE_POS * Bb),
            np.ascontiguousarray(xto).reshape(128, NO_POS * Bb))


# ------------------------------------------------------------ program build
def build_program():
    nc = bacc.Bacc("TRN2", target_bir_lowering=False, debug=False)

    xte_d = nc.dram_tensor("xte", [128, NE_POS * Bc], F32, kind="ExternalInput")
    xto_d = nc.dram_tensor("xto", [128, NO_POS * Bc], F32, kind="ExternalInput")
    wa_d = nc.dram_tensor("wallA", [128, 96], F32, kind="ExternalInput")
    wb_d = nc.dram_tensor("wallB", [128, 4 * 96], F32, kind="ExternalInput")
    u_d = nc.dram_tensor("u_out", [Bc, 3, T_FULL], F32, kind="ExternalOutput")
    s_d = nc.dram_tensor("s_out", [Bc, 3, T_FULL], F32, kind="ExternalOutput")

    with tile.TileContext(nc) as tc:
        with (
            tc.tile_pool(name="const", bufs=1) as constp,
            tc.tile_pool(name="xbuf", bufs=1) as xbuf,
            tc.tile_pool(name="wave", bufs=1) as wave,
            tc.tile_pool(name="state", bufs=1) as state,
            tc.tile_pool(name="psC", bufs=8, space="PSUM") as psC,
        ):
            wa_sb = constp.tile([128, 96], F32, tag="wa")
            wb_sb = constp.tile([128, 4 * 96], F32, tag="wb")
            xTe = xbuf.tile([128, NE_POS, Bc], F32, tag="xTe")
            xTo = xbuf.tile([128, NO_POS, Bc], F32, tag="xTo")
            nc.sync.dma_start(wa_sb[:], wa_d.ap())
            nc.sync.dma_start(wb_sb[:], wb_d.ap())
            # split strip loads so early matmuls can start promptly
            nxd = 4
            for i in range(nxd):
                el = NE_POS * Bc // nxd
                ol = NO_POS * Bc // nxd
                nc.sync.dma_start(xTe[:, 0, 0].tensor_slice2(i * el, el),
                                  xte_d.ap()[:, i * el:(i + 1) * el]) \
                    if False else None
                e0 = xTe[:, 0, :]
                nc.sync.dma_start(
                    bass.AP(e0.tensor, e0.offset + i * el, [e0.ap[0], [1, el]]),
                    xte_d.ap()[:, i * el:(i + 1) * el])
                o0 = xTo[:, 0, :]
                nc.sync.dma_start(
                    bass.AP(o0.tensor, o0.offset + i * ol, [o0.ap[0], [1, ol]]),
                    xto_d.ap()[:, i * ol:(i + 1) * ol])

            def xt_block(c2p, rows=None):
                """Stationary AP for block {c2p, c2p+32, c2p+64, c2p+96}."""
                if c2p % 2 == 0:
                    strip, pos = xTe, EVEN_POS[c2p]
                else:
                    strip, pos = xTo, ODD_POS[c2p]
                a = strip[:, 0, :] if rows is None else strip[rows[0]:rows[1], 0, :]
                return bass.AP(a.tensor, a.offset + pos * Bc,
                               [a.ap[0], [1, 4 * Bc]])

            # wavefront tiles: [128, Q(jq), NC2(c2), 4 lanes (3 used)]
            uq = [wave.tile([128, NC2, 3, Q], F32, tag=f"uq{q}", name=f"uq{q}")
                  for q in range(NQ)]
            vq = [wave.tile([128, Q, NC2, 4], F32, tag=f"vq{q}", name=f"vq{q}")
                  for q in range(NQ)]
            g = state.tile([128, NC2], F32, tag="g")
            binit = state.tile([128, 4], F32, tag="binit")
            vpre0 = state.tile([128, NC2, 4], F32, tag="vpre0")
            nc.vector.memset(binit[:], 0.0)

            def g_b():
                ga = g[:, :]
                return bass.AP(ga.tensor, ga.offset, list(ga.ap) + [[0, 3]])

            # ---------------- conv: quarter-major matmuls, partition-aligned
            # group g = c2: windows {32*cs + c2}; pc partitions = 32*cs + b
            # align 1:1 with uq partitions -> one [128, 96] copy per
            # (quarter, group).
            for q in range(NQ):
                for c2 in range(NC2):
                    pc = psC.tile([128, 96], F32, tag="psC")
                    lhsB = xt_block(c2 + 1)
                    if q == 0:
                        lhsA = xt_block(c2, rows=(64, 128))
                        nc.tensor.matmul(pc[:], lhsB, wb_sb[:, 0:96],
                                         start=True, stop=False)
                        nc.tensor.matmul(pc[:], lhsA, wa_sb[64:128, :],
                                         start=False, stop=True)
                    else:
                        nc.tensor.matmul(pc[:], lhsB,
                                         wb_sb[:, 96 * q:96 * (q + 1)],
                                         start=True, stop=True)
                    d0 = uq[q][:, c2, 0, 0]
                    dst = bass.AP(d0.tensor, d0.offset, [d0.ap[0], [1, 96]])
                    nc.scalar.copy(dst, pc[:])

            # ---------------- u DMA out: t = 128*(cs*NC2+c2) + q*32 + jq
            for cs in range(CS):
                for q in range(NQ):
                    for k in range(3):
                        src = uq[q][Bc * cs:Bc * (cs + 1), :, k, :]
                        dst_ap = bass.AP(
                            u_d.ap().tensor,
                            (k * T_FULL + cs * NC2 * C + q * Q),
                            [[3 * T_FULL, Bc], [C, NC2], [1, Q]])
                        nc.sync.dma_start(dst_ap, src)

            # ---------------- LIF wavefront
            def vsl(sigma, c2a=0, c2b=NC2):
                q, jq = divmod(sigma, Q)
                return vq[q][:, jq, c2a:c2b, 0:3]

            def usl(sigma, c2a=0, c2b=NC2):
                q, jq = divmod(sigma, Q)
                return uq[q][:, c2a:c2b, :, jq]

            def recover_quarter(q):
                """s = (alpha*v[t-1] + u[t]) - v[t] for quarter q, written
                over uq[q] (Pool engine), then DMA s out."""
                if q == 0:
                    nc.vector.tensor_copy(uq[0][:, :, :, 0], vpre0[:, :, 0:3])
                else:
                    nc.vector.scalar_tensor_tensor(
                        uq[q][:, :, :, 0], vq[q - 1][:, Q - 1, :, 0:3],
                        float(ALPHA), uq[q][:, :, :, 0],
                        op0=A.mult, op1=A.add)
                for k in range(3):
                    v0 = vq[q][:, 0, 0, k]
                    vslab0 = bass.AP(v0.tensor, v0.offset,
                                     [v0.ap[0], [4, NC2], [4 * NC2, Q - 1]])
                    vslab = bass.AP(v0.tensor, v0.offset,
                                    [v0.ap[0], [4, NC2], [4 * NC2, Q]])
                    u0 = uq[q][:, 0, k, 0]
                    uslab1 = bass.AP(u0.tensor, u0.offset + 1,
                                     [u0.ap[0], [3 * Q, NC2], [1, Q - 1]])
                    uslab = bass.AP(u0.tensor, u0.offset,
                                    [u0.ap[0], [3 * Q, NC2], [1, Q]])
                    nc.vector.scalar_tensor_tensor(
                        uslab1, vslab0, float(ALPHA), uslab1,
                        op0=A.mult, op1=A.add)
                    nc.vector.tensor_tensor(uslab, uslab, vslab,
                                            op=A.subtract)
                for cs in range(CS):
                    for k in range(3):
                        src = uq[q][Bc * cs:Bc * (cs + 1), :, k, :]
                        dst_ap = bass.AP(
                            s_d.ap().tensor,
                            (k * T_FULL + cs * NC2 * C + q * Q),
                            [[3 * T_FULL, Bc], [C, NC2], [1, Q]])
                        nc.sync.dma_start(dst_ap, src)

            for p in range(P):
                if p == P - 1:
                    # quarter-3 spikes come from pass P-1's trajectory, which
                    # the final (partial) pass never touches; recover now.
                    for qq in range(H // Q, NQ):
                        recover_quarter(qq)
                if p > 0:
                    # binit rows 32..127 <- prev-pass end of last chunk of
                    # previous slot (partition-shifted copy); rows 0..31 stay 0
                    for cs in range(1, CS):
                        bsrc0 = vq[NQ - 1][Bc * (cs - 1):Bc * cs,
                                           Q - 1, NC2 - 1, 0:3]
                        bsrc = bass.AP(bsrc0.tensor, bsrc0.offset,
                                       [bsrc0.ap[0], [1, 3]])
                        nc.vector.tensor_copy(binit[Bc * cs:Bc * (cs + 1), 0:3],
                                              bsrc)
                for sigma in range(H if p == P - 1 else C):
                    cur = vsl(sigma)
                    if sigma == 0:
                        if p == 0:
                            nc.vector.tensor_copy(cur, usl(0))
                        else:
                            nc.vector.scalar_tensor_tensor(
                                vsl(0, 1, NC2), vq[NQ - 1][:, Q - 1, 0:NC2 - 1, 0:3],
                                float(ALPHA), usl(0, 1, NC2),
                                op0=A.mult, op1=A.add)
                            bi = binit[:, 0:3]
                            bi3 = bass.AP(bi.tensor, bi.offset,
                                          [bi.ap[0], [0, 1], [1, 3]])
                            nc.vector.scalar_tensor_tensor(
                                vsl(0, 0, 1), bi3, float(ALPHA), usl(0, 0, 1),
                                op0=A.mult, op1=A.add)
                    else:
                        nc.vector.scalar_tensor_tensor(
                            cur, vsl(sigma - 1), float(ALPHA), usl(sigma),
                            op0=A.mult, op1=A.add)
                    if p == P - 1 and sigma == 0:
                        nc.vector.tensor_copy(vpre0[:, :, 0:3], cur)
                    nc.vector.tensor_reduce(
                        g[:, :], cur, axis=mybir.AxisListType.X, op=A.max)
                    nc.vector._custom_dve(LIF_RESET, out=cur, in0=cur,
                                          in1=g_b(), s0=1.0)
                    if p == P - 1 and sigma % Q == Q - 1:
                        recover_quarter(sigma // Q)

            # (recovery + s DMA are emitted per-quarter inside the final
            # pass via recover_quarter)

    nc.compile()
    return nc


# ----------------------------------------------------------------- running
def _ensure_ntff_hook():
    """Register the axon NTFF profiling hook."""
    import types
    try:
        from antenv.axon_hooks import get_axon_ntff_profile_hook  # noqa: F401
        return
    except ImportError:
        pass
    import antenv
    mod = types.ModuleType("antenv.axon_hooks")
    _state = {"hook": None}
    mod.set_axon_ntff_profile_hook = lambda h: _state.__setitem__("hook", h)
    mod.get_axon_ntff_profile_hook = lambda: _state["hook"]
    sys.modules["antenv.axon_hooks"] = mod
    antenv.axon_hooks = mod
    try:
        from trn_agent_boot.trn_boot import _ntff_profile_via_ctypes
        hook = _ntff_profile_via_ctypes("/opt/axon/libaxon_pjrt.so")
        if hook is not None:
            mod.set_axon_ntff_profile_hook(hook)
    except Exception as e:  # profiling optional
        print(f"ntff hook unavailable: {e}", file=sys.stderr)


_CACHE = {}


def _get_program():
    if "p" not in _CACHE:
        _CACHE["p"] = build_program()
    return _CACHE["p"]


def kernel(x, w0, w1, w2, y=None, trace=False):
    x = np.asarray(x, np.float32)
    ws = [np.asarray(w, np.float32).reshape(-1) for w in (w0, w1, w2)]
    B = x.shape[0]
    assert B == B_FULL and x.shape[-1] == T_FULL

    wallA, wallB = build_walls(ws)
    xts = [build_xt(x.reshape(B, T_FULL)[c * Bc:(c + 1) * Bc])
           for c in range(N_CORES)]

    if trace:
        _ensure_ntff_hook()
    nc = _get_program()
    in_maps = [
        {"xte": xts[c][0], "xto": xts[c][1], "wallA": wallA, "wallB": wallB}
        for c in range(N_CORES)
    ]
    res = run_bass_kernel_spmd(nc, in_maps, core_ids=list(range(N_CORES)),
                               trace=trace)
    u = np.concatenate([r["u_out"] for r in res.results], axis=0)
    s = np.concatenate([r["s_out"] for r in res.results], axis=0)
    if trace:
        kernel.last_exec_time_ns = res.exec_time_ns
    return (u, s)


kernel.last_exec_time_ns = None
